# revision 1
# baseline (speedup 1.0000x reference)
"""Trainium2 Bass kernel for nn_DecoderLayer_33758442946809.

Sharding (8 cores = 2 batches x 4-core groups):
- Self-attention is HEAD-sharded: core (b, j) computes heads 4j..4j+3 for
  all T=2048 rows of batch b. This removes the K/V-duplication of pure
  data-parallelism (each core projects Q/K/V only for its own 4 heads) and
  makes causal skipping SPMD-uniform: score/attnV/exp work is done only
  for lower-triangle key blocks (40 of 64 per head).
- W1 is row-parallel (contraction over the head-sharded cat features);
  partials are summed with a chunked ReduceScatter (bf16) over each
  4-core group, pipelined under the attention of the next row chunk.
  After the RS, core (b, j) owns the strided row set
  {512*rc + 128*j + i : rc<4, i<128} and everything downstream
  (cross-attention, W2, FFN) is data-parallel over those rows.
- Score matmuls (K=DK=64) are row-tiled: both heads of a pair run
  concurrently in the PE array at tile_position (0,0)/(64,0), writing the
  two halves of one [128,1024] PSUM tile, so exp runs as one N=1024
  ACT instruction per key block.
- Softmax uses no max-subtraction (logits bounded); denominators ride as
  a ones-column in V; reciprocal rows are broadcast across partitions via
  a K=1 PE matmul (keeps GpSimd free for the collectives).
- tile(attn2, H) @ W2 == attn2 @ sum_h W2[h] (host precomputes the sum).
"""
import math
import sys

import numpy as np

sys.path.insert(0, "/opt/trn_rl_repo")

import ml_dtypes  # noqa: E402

import concourse.bass as bass  # noqa: E402
import concourse.tile as tile  # noqa: E402
from concourse import bacc, mybir  # noqa: E402
from concourse.bass_utils import run_bass_kernel_spmd  # noqa: E402
from concourse.masks import make_identity  # noqa: E402

B, S, D, H, DF = 2, 2048, 1024, 16, 4096
DK = D // H                      # 64
P = 128
T = S                            # rows/keys per batch
R = 512                          # own rows per core (after RS)
KC = D // P                      # 8 contraction chunks of D
TB = T // P                      # 16 key blocks
RB = R // P                      # 4 row blocks
FB = DF // P                     # 32 ffn blocks
NCORES = 8
HL = 4                           # local heads per core
SCALE = 1.0 / math.sqrt(DK)
RG = [[0, 1, 2, 3], [4, 5, 6, 7]]

F32 = mybir.dt.float32
BF16 = mybir.dt.bfloat16
AF = mybir.ActivationFunctionType
ALU = mybir.AluOpType

_cached = {}


def _ln_rows(nc, pool, x_ap, eps_sb, g_b, be_b):
    """In-place LayerNorm along the free axis (D) of a token-major
    [128, D] f32 tile, with per-feature affine from broadcast tiles."""
    x3 = x_ap.rearrange("p (n f) -> p n f", f=512)
    stats = pool.tile([P, 2, 6], F32, name="ln_stats", tag="ln_stats", bufs=4)
    for sg in range(2):
        nc.vector.bn_stats(out=stats[:, sg, :], in_=x3[:, sg, :])
    mv = pool.tile([P, 2], F32, name="ln_mv", tag="ln_mv", bufs=4)
    nc.vector.bn_aggr(out=mv[:], in_=stats[:])
    std = pool.tile([P, 1], F32, name="ln_std", tag="ln_std", bufs=4)
    nc.scalar.activation(out=std[:], in_=mv[:, 1:2], func=AF.Sqrt,
                         bias=eps_sb[:], scale=1.0)
    rstd = pool.tile([P, 1], F32, name="ln_rstd", tag="ln_rstd", bufs=4)
    nc.vector.reciprocal(out=rstd[:], in_=std[:])
    nc.vector.tensor_scalar(out=x_ap, in0=x_ap, scalar1=mv[:, 0:1],
                            scalar2=rstd[:], op0=ALU.subtract, op1=ALU.mult)
    nc.vector.tensor_mul(out=x_ap, in0=x_ap, in1=g_b)
    nc.vector.tensor_add(out=x_ap, in0=x_ap, in1=be_b)


def build_nc():
    nc = bacc.Bacc("TRN2", target_bir_lowering=False, debug=False,
                   num_devices=NCORES)

    dram = {}

    def din(name, shape, dt):
        dram[name] = nc.dram_tensor(name, shape, dt, kind="ExternalInput").ap()

    din("yT", [D, T], BF16)          # y[b].T
    din("wq", [P, KC * HL * DK], BF16)   # pre-chunked [p][kc][f] layout
    din("wk", [P, KC * HL * DK], BF16)
    din("wv", [P, KC * HL * DK], BF16)
    din("bq_s", [HL * DK], F32)      # bq * SCALE, local heads
    din("bk_f", [HL * DK], F32)
    din("bv_f", [HL * DK], F32)
    din("mask4", [P, 4, R], BF16)    # diagonal-block masks (key, i, row)
    din("w1loc", [P, 2 * D], BF16)   # W1 rows owned by this core, pre-chunked
    din("b1", [D], F32)
    din("ln1_g", [D], F32)
    din("ln1_b", [D], F32)
    din("y_rows", [R, D], F32)       # this core's (strided) y rows
    din("xT", [D, T], BF16)
    din("x_tm", [T, D], BF16)
    din("W2sum", [D, D], BF16)
    din("b2", [D], F32)
    din("ln2_g", [D], F32)
    din("ln2_b", [D], F32)
    din("Wf1", [P, KC * DF], BF16)   # pre-chunked [p][kc][f] layout
    din("bf1", [DF], F32)
    din("Wf2", [DF, D], BF16)
    din("bf2", [D], F32)
    din("ln3_g", [D], F32)
    din("ln3_b", [D], F32)
    out_d = nc.dram_tensor("out", [R, D], F32, kind="ExternalOutput").ap()

    with tile.TileContext(nc) as tc:
        _build(nc, tc, dram, out_d)
    nc.compile()
    return nc


def _build(nc, tc, d, out_d):
    pool_cms = {}

    def open_pool(*args, **kw):
        cm = tc.tile_pool(*args, **kw)
        p = cm.__enter__()
        pool_cms[id(p)] = cm
        return p

    def close_pool(p):
        pool_cms.pop(id(p)).__exit__(None, None, None)

    const = open_pool(name="const", bufs=1, side="left")
    ident = const.tile([P, P], F32, name="ident", tag="ident")
    make_identity(nc, ident[:])
    ones_col = const.tile([P, 1], BF16, name="ones_col", tag="ones_col")
    nc.vector.memset(ones_col[:], 1.0)
    ones_row = const.tile([1, P], BF16, name="ones_row", tag="ones_row")
    nc.vector.memset(ones_row[:], 1.0)
    eps_sb = const.tile([P, 1], F32, name="eps", tag="eps")
    nc.vector.memset(eps_sb[:], 1e-5)

    def bias_chunks(pool, name, n):
        t = pool.tile([P, n], F32, name=f"bc_{name}", tag=f"bc_{name}")
        nc.sync.dma_start(out=t[:], in_=d[name].rearrange("(n p) -> p n", p=P))
        return t

    def bcast_row(pool, name):
        src = d[name]
        t = pool.tile([P, D], F32, name=f"br_{name}", tag=f"br_{name}")
        bc = bass.AP(tensor=src.tensor, offset=src.offset,
                     ap=[[0, P]] + list(src.ap))
        nc.sync.dma_start(out=t[:], in_=bc)
        return t

    # ===================== Phase 1: QKV projections =====================
    ph1 = open_pool(name="ph1", bufs=1, side="left")
    attn = open_pool(name="attn", bufs=1, side="right")  # live through ph2
    qTp = [attn.tile([P, T], BF16, name=f"qTp{i}", tag=f"qTp{i}") for i in range(2)]
    kTp = [attn.tile([P, T], BF16, name=f"kTp{i}", tag=f"kTp{i}") for i in range(2)]
    v_sb = [attn.tile([P, HL, DK + 1], BF16, name=f"v{i}", tag=f"v{i}")
            for i in range(TB)]
    mask4 = attn.tile([P, 4, R], BF16, name="mask4", tag="mask4")
    nc.sync.dma_start(out=mask4[:], in_=d["mask4"][:])

    yT = [ph1.tile([P, T], BF16, name=f"yT{i}", tag=f"yT{i}") for i in range(KC)]
    wq_sb = ph1.tile([P, KC, 2 * P], BF16, name="wq", tag="wq")
    wk_sb = ph1.tile([P, KC, 2 * P], BF16, name="wk", tag="wk")
    wv_sb = ph1.tile([P, KC, 2 * P], BF16, name="wv", tag="wv")
    nc.scalar.dma_start(out=wq_sb[:],
                        in_=d["wq"].rearrange("p (c f) -> p c f", c=KC))
    nc.scalar.dma_start(out=wk_sb[:],
                        in_=d["wk"].rearrange("p (c f) -> p c f", c=KC))
    nc.gpsimd.dma_start(out=wv_sb[:],
                        in_=d["wv"].rearrange("p (c f) -> p c f", c=KC))
    for kc in range(KC):
        nc.sync.dma_start(out=yT[kc][:], in_=d["yT"][kc * P:(kc + 1) * P, :])
    bq_sb = bias_chunks(ph1, "bq_s", 2)
    bk_sb = bias_chunks(ph1, "bk_f", 2)
    bv_b = ph1.tile([P, 2 * P], F32, name="bv_b", tag="bv_b")
    bv_src = d["bv_f"]
    nc.sync.dma_start(out=bv_b[:], in_=bass.AP(
        tensor=bv_src.tensor, offset=bv_src.offset,
        ap=[[0, P]] + list(bv_src.ap)))

    ps1 = open_pool(name="ps1", bufs=4, space="PSUM", side="left")
    for p in range(2):
        for tcol in range(T // 512):
            ps = ps1.tile([P, 512], F32, name="psq", tag="psq")
            for kc in range(KC):
                nc.tensor.matmul(ps[:], lhsT=wq_sb[:, kc, p * P:(p + 1) * P],
                                 rhs=yT[kc][:, tcol * 512:(tcol + 1) * 512],
                                 start=(kc == 0), stop=(kc == KC - 1))
            nc.vector.tensor_scalar(out=qTp[p][:, tcol * 512:(tcol + 1) * 512],
                                    in0=ps[:], scalar1=bq_sb[:, p:p + 1],
                                    scalar2=None, op0=ALU.add)
        for tcol in range(T // 512):
            ps = ps1.tile([P, 512], F32, name="psq", tag="psq")
            for kc in range(KC):
                nc.tensor.matmul(ps[:], lhsT=wk_sb[:, kc, p * P:(p + 1) * P],
                                 rhs=yT[kc][:, tcol * 512:(tcol + 1) * 512],
                                 start=(kc == 0), stop=(kc == KC - 1))
            nc.vector.tensor_scalar(out=kTp[p][:, tcol * 512:(tcol + 1) * 512],
                                    in0=ps[:], scalar1=bk_sb[:, p:p + 1],
                                    scalar2=None, op0=ALU.add)
    for tb in range(TB):
        nc.vector.memset(v_sb[tb][:, :, DK:DK + 1], 1.0)
        ps = ps1.tile([P, 2 * P], F32, name="psv", tag="psv", bufs=2)
        for kc in range(KC):
            nc.tensor.matmul(ps[:], lhsT=yT[kc][:, tb * P:(tb + 1) * P],
                             rhs=wv_sb[:, kc, :],
                             start=(kc == 0), stop=(kc == KC - 1))
        nc.vector.tensor_add(
            out=v_sb[tb][:, :, 0:DK],
            in0=ps[:].rearrange("p (h k) -> p h k", h=HL),
            in1=bv_b[:].rearrange("p (h k) -> p h k", h=HL))
    close_pool(ps1)
    close_pool(ph1)

    # ============ Phase 2+3: causal attention + W1 + ReduceScatter ======
    cat = open_pool(name="cat", bufs=1, side="right")     # live into ph3
    catT = [cat.tile([P, T], BF16, name=f"catT{i}", tag=f"catT{i}")
            for i in range(2)]
    ph3 = open_pool(name="ph3", bufs=1, side="right")
    w1_sb = ph3.tile([P, 2, D], BF16, name="w1", tag="w1")
    nc.sync.dma_start(out=w1_sb[:], in_=d["w1loc"].rearrange("p (c n) -> p c n", c=2))
    a1pre = [ph3.tile([P, D], BF16, name=f"a1pre{i}", tag=f"a1pre{i}")
             for i in range(RB)]

    dramp = open_pool(name="dramp", bufs=1, space="DRAM", side="left")
    rs_in = [dramp.tile([4 * P, D], BF16, name=f"rsi{i}", tag=f"rsi{i}")
             for i in range(RB)]
    rs_out = [dramp.tile([P, D], BF16, name=f"rso{i}", tag=f"rso{i}")
              for i in range(RB)]

    ph2 = open_pool(name="ph2", bufs=1, side="left")
    psS = open_pool(name="psS", bufs=1, space="PSUM", side="left")
    psV = open_pool(name="psV", bufs=1, space="PSUM", side="left")
    psW = open_pool(name="psW", bufs=1, space="PSUM", side="right")

    # Work from the previous slot (attnV matmuls, softmax evictions, W1 +
    # ReduceScatter) is emitted as callbacks interleaved between the next
    # slot's score matmuls, so the in-order PE queue always has runnable
    # work while ACT exp (the phase bottleneck) lags behind.
    pending = []

    def drain(k):
        for _ in range(min(k, len(pending))):
            pending.pop(0)()

    def make_attn_work(rc, p, expP):
        nkb = 4 * rc + 4
        work = []
        pa_t = [None, None]
        attn_sb = [None, None]

        def start_head(hh):
            pa_t[hh] = psV.tile([DK + 1, 512], F32, name="pa", tag="pa",
                                bufs=3)

        def mm_head(hh, kb):
            hl = 2 * p + hh
            nc.tensor.matmul(pa_t[hh][:], lhsT=v_sb[kb][:, hl, :],
                             rhs=expP[:, kb, hh, :],
                             start=(kb == 0), stop=(kb == nkb - 1))

        den_row = [None]

        def evict_head(hh):
            attn_sb[hh] = ph2.tile([DK + 1, 512], BF16, name="attn_sb",
                                   tag="attn_sb", bufs=3)
            nc.vector.tensor_copy(out=attn_sb[hh][:], in_=pa_t[hh][:])
            if hh == 0:
                den_row[0] = ph2.tile([1, 2, 512], BF16, name="den_row",
                                      tag="den_row", bufs=2)
            nc.vector.tensor_copy(out=den_row[0][:, hh, :],
                                  in_=pa_t[hh][DK:DK + 1, :])

        def finish_pair():
            denB = ph2.tile([DK, 2, 512], BF16, name="denB", tag="denB", bufs=2)
            for hh in range(2):
                nc.gpsimd.partition_broadcast(
                    denB[:, hh, :], den_row[0][:, hh, :])
            recB = ph2.tile([DK, 2, 512], BF16, name="recB", tag="recB", bufs=2)
            with nc.allow_low_precision(reason="softmax denom bf16 ok"):
                nc.vector.reciprocal(out=recB[:], in_=denB[:])
            for hh in range(2):
                nc.vector.tensor_mul(
                    out=catT[p][hh * DK:(hh + 1) * DK,
                                rc * 512:(rc + 1) * 512],
                    in0=attn_sb[hh][0:DK, :], in1=recB[:, hh, :])

        for hh in range(2):
            work.append(lambda hh=hh: start_head(hh))
            for kb in range(nkb):
                work.append(lambda hh=hh, kb=kb: mm_head(hh, kb))
            work.append(lambda hh=hh: evict_head(hh))
        work.append(finish_pair)
        return work

    def make_w1_work(rc):
        work = []

        def w1_block(rb, nt, box):
            c0 = rc * 512 + rb * P
            if nt == 0:
                box[0] = ph2.tile([P, D], BF16, name="a1p", tag="a1p", bufs=2)
            psw = psW.tile([P, 512], F32, name="psw", tag="psw", bufs=1)
            for kc2 in range(2):
                nc.tensor.matmul(psw[:],
                                 lhsT=catT[kc2][:, c0:c0 + P],
                                 rhs=w1_sb[:, kc2, nt * 512:(nt + 1) * 512],
                                 start=(kc2 == 0), stop=(kc2 == 1))
            nc.vector.tensor_copy(out=box[0][:, nt * 512:(nt + 1) * 512],
                                  in_=psw[:])
            if nt == 1:
                nc.gpsimd.dma_start(out=rs_in[rc][rb * P:(rb + 1) * P, :],
                                    in_=box[0][:])

        for rb in range(4):
            box = [None]
            for nt in range(2):
                work.append(lambda rb=rb, nt=nt, box=box: w1_block(rb, nt, box))

        def do_rs():
            nc.gpsimd.collective_compute(
                "ReduceScatter", ALU.add, replica_groups=RG,
                ins=[rs_in[rc][:].opt()], outs=[rs_out[rc][:].opt()])
            nc.gpsimd.dma_start(out=a1pre[rc][:], in_=rs_out[rc][:])
        work.append(do_rs)
        return work

    for rc in range(RB):
        nkb = 4 * rc + 4
        for p in range(2):
            # expP[kb][i] holds exp(scores) for head 2p+i, keys block kb
            expP = ph2.tile([P, TB, 2, 512], BF16, name="expP", tag="expP",
                            bufs=2)
            ngrp = nkb // 2
            for g in range(ngrp):
                ps = psS.tile([P, 2048], F32, name="ps_sc", tag="ps_sc")
                for u in range(2):
                    kb = 2 * g + u
                    nc.tensor.matmul(ps[:, u * 1024:u * 1024 + 512],
                                     lhsT=kTp[p][0:DK, kb * P:(kb + 1) * P],
                                     rhs=qTp[p][0:DK, rc * 512:(rc + 1) * 512],
                                     start=True, stop=True,
                                     tile_position=(0, 0))
                    nc.tensor.matmul(ps[:, u * 1024 + 512:(u + 1) * 1024],
                                     lhsT=kTp[p][DK:P, kb * P:(kb + 1) * P],
                                     rhs=qTp[p][DK:P, rc * 512:(rc + 1) * 512],
                                     start=True, stop=True,
                                     tile_position=(64, 0))
                    drain(3)
                nc.scalar.activation(
                    out=expP[:, 2 * g:2 * g + 2, :, :],
                    in_=ps[:].rearrange("p (a h r) -> p a h r", a=2, h=2),
                    func=AF.Exp)
                for u in range(2):
                    kb = 2 * g + u
                    if kb >= 4 * rc:       # diagonal block: apply causal mask
                        i = kb - 4 * rc
                        for hh in range(2):
                            nc.vector.tensor_mul(out=expP[:, kb, hh, :],
                                                 in0=expP[:, kb, hh, :],
                                                 in1=mask4[:, i, :])
            pending.extend(make_attn_work(rc, p, expP))
        pending.extend(make_w1_work(rc))
    drain(len(pending))
    close_pool(psV)
    close_pool(psS)
    close_pool(ph2)

    # ============== Phase 3b: residual + LN1, produce a1T ===============
    a1pl = open_pool(name="a1pl", bufs=1, side="left")    # live through ph4
    a1T = [a1pl.tile([P, R], BF16, name=f"a1T{i}", tag=f"a1T{i}")
           for i in range(KC)]
    psT = open_pool(name="psT", bufs=2, space="PSUM", side="left")
    y_sb = [ph3.tile([P, D], F32, name=f"y{i}", tag=f"y{i}") for i in range(RB)]
    for rb in range(RB):
        nc.sync.dma_start(out=y_sb[rb][:], in_=d["y_rows"][rb * P:(rb + 1) * P, :])
    b1_b = bcast_row(ph3, "b1")
    g1_b = bcast_row(ph3, "ln1_g")
    be1_b = bcast_row(ph3, "ln1_b")
    for rc in range(RB):
        a1 = ph3.tile([P, D], F32, name="a1", tag="a1", bufs=2)
        nc.vector.tensor_copy(out=a1[:], in_=a1pre[rc][:])
        nc.vector.tensor_add(out=a1[:], in0=a1[:], in1=y_sb[rc][:])
        nc.vector.tensor_add(out=a1[:], in0=a1[:], in1=b1_b[:])
        _ln_rows(nc, ph3, a1[:], eps_sb, g1_b[:], be1_b[:])
        for kc in range(KC):
            pt = psT.tile([P, P], F32, name="pt", tag="pt")
            nc.tensor.transpose(pt[:], a1[:, kc * P:(kc + 1) * P], ident[:])
            nc.vector.tensor_scalar(out=a1T[kc][:, rc * P:(rc + 1) * P],
                                    in0=pt[:], scalar1=float(SCALE),
                                    scalar2=None, op0=ALU.mult)
    close_pool(psT)
    close_pool(psW)
    close_pool(ph3)
    close_pool(cat)
    close_pool(attn)

    # ================= Phase 4: cross-attention =========================
    at2p = open_pool(name="at2p", bufs=1, side="right")   # at2T live into ph5
    at2T = [at2p.tile([P, R], BF16, name=f"at2T{i}", tag=f"at2T{i}")
            for i in range(KC)]
    ph4 = open_pool(name="ph4", bufs=1, side="left")
    pp4 = open_pool(name="pp4", bufs=4, space="PSUM", side="left")
    pd4 = open_pool(name="pd4", bufs=1, space="PSUM", side="left")
    xT = [ph4.tile([P, T], BF16, name=f"xT{i}", tag=f"xT{i}") for i in range(KC)]
    for kc in range(KC):
        nc.sync.dma_start(out=xT[kc][:], in_=d["xT"][kc * P:(kc + 1) * P, :])
    x_tm = [ph4.tile([P, D], BF16, name=f"xtm{i}", tag=f"xtm{i}")
            for i in range(TB)]
    for tb in range(TB):
        nc.sync.dma_start(out=x_tm[tb][:], in_=d["x_tm"][tb * P:(tb + 1) * P, :])
    p2T = [ph4.tile([P, R], BF16, name=f"p2T{i}", tag=f"p2T{i}")
           for i in range(TB)]
    for tb in range(TB):
        ps = pp4.tile([P, 512], F32, name="ps4", tag="ps4")
        for kc in range(KC):
            nc.tensor.matmul(ps[:], lhsT=xT[kc][:, tb * P:(tb + 1) * P],
                             rhs=a1T[kc][:, :],
                             start=(kc == 0), stop=(kc == KC - 1))
        nc.scalar.activation(out=p2T[tb][:], in_=ps[:], func=AF.Exp)
    pd = pd4.tile([1, R], F32, name="ps_d2", tag="ps_d2")
    for tb in range(TB):
        nc.tensor.matmul(pd[:], lhsT=ones_col[:], rhs=p2T[tb][:],
                         start=(tb == 0), stop=(tb == TB - 1))
    recip2 = ph4.tile([1, R], BF16, name="recip2", tag="recip2")
    with nc.allow_low_precision(reason="softmax denom bf16 ok"):
        nc.vector.reciprocal(out=recip2[:], in_=pd[:])
    psb2 = pd4.tile([P, R], F32, name="psb2", tag="psb2")
    nc.tensor.matmul(psb2[:], lhsT=ones_row[:], rhs=recip2[:],
                     start=True, stop=True)
    recip2b = ph4.tile([P, R], F32, name="recip2b", tag="recip2b")
    nc.vector.tensor_copy(out=recip2b[:], in_=psb2[:])
    for db in range(KC):
        ps = pp4.tile([P, 512], F32, name="ps4", tag="ps4")
        for tb in range(TB):
            nc.tensor.matmul(ps[:], lhsT=x_tm[tb][:, db * P:(db + 1) * P],
                             rhs=p2T[tb][:],
                             start=(tb == 0), stop=(tb == TB - 1))
        nc.vector.tensor_mul(out=at2T[db][:], in0=ps[:], in1=recip2b[:])
    close_pool(pd4)
    close_pool(pp4)
    close_pool(ph4)
    close_pool(a1pl)

    # ========= Phase 5: W2sum + residual + LN2, produce a2T =============
    a2p = open_pool(name="a2p", bufs=1, side="left")      # a2T live into ph6
    a2T = [a2p.tile([P, R], BF16, name=f"a2T{i}", tag=f"a2T{i}")
           for i in range(KC)]
    fw = open_pool(name="fw", bufs=1, side="left")        # Wf1 (prefetched)
    wf1_all = fw.tile([P, KC, DF], BF16, name="wf1", tag="wf1")
    nc.gpsimd.dma_start(out=wf1_all[:],
                        in_=d["Wf1"].rearrange("p (c f) -> p c f", c=KC))

    ph5 = open_pool(name="ph5", bufs=1, side="right")
    pp5 = open_pool(name="pp5", bufs=4, space="PSUM", side="right")
    pt5 = open_pool(name="pt5", bufs=2, space="PSUM", side="right")
    w2 = [ph5.tile([P, D], BF16, name=f"w2_{i}", tag=f"w2_{i}")
          for i in range(KC)]
    y_sb5 = [ph5.tile([P, D], F32, name=f"y5{i}", tag=f"y5{i}")
             for i in range(RB)]
    for kc in range(KC):
        nc.sync.dma_start(out=w2[kc][:], in_=d["W2sum"][kc * P:(kc + 1) * P, :])
    for rb in range(RB):
        nc.sync.dma_start(out=y_sb5[rb][:], in_=d["y_rows"][rb * P:(rb + 1) * P, :])
    b2_b = bcast_row(ph5, "b2")
    g2_b = bcast_row(ph5, "ln2_g")
    be2_b = bcast_row(ph5, "ln2_b")
    for rb in range(RB):
        a2 = ph5.tile([P, D], F32, name="a2", tag="a2", bufs=2)
        for nt in range(2):
            ps = pp5.tile([P, 512], F32, name="ps_a2", tag="ps_a2")
            for kc in range(KC):
                nc.tensor.matmul(ps[:],
                                 lhsT=at2T[kc][:, rb * P:(rb + 1) * P],
                                 rhs=w2[kc][:, nt * 512:(nt + 1) * 512],
                                 start=(kc == 0), stop=(kc == KC - 1))
            sl = slice(nt * 512, (nt + 1) * 512)
            nc.vector.tensor_add(out=a2[:, sl], in0=ps[:], in1=y_sb5[rb][:, sl])
            nc.vector.tensor_add(out=a2[:, sl], in0=a2[:, sl], in1=b2_b[:, sl])
        _ln_rows(nc, ph5, a2[:], eps_sb, g2_b[:], be2_b[:])
        for kc in range(KC):
            pt = pt5.tile([P, P], F32, name="pt_a2", tag="pt_a2")
            nc.tensor.transpose(pt[:], a2[:, kc * P:(kc + 1) * P], ident[:])
            nc.vector.tensor_copy(out=a2T[kc][:, rb * P:(rb + 1) * P],
                                  in_=pt[:])
    close_pool(pt5)
    close_pool(pp5)
    close_pool(ph5)
    close_pool(at2p)

    # ========== Phase 6: FFN + residual + LN3 ===========================
    fA = open_pool(name="fA", bufs=1, side="right")
    f1T = [fA.tile([P, R], BF16, name=f"f1T{i}", tag=f"f1T{i}")
           for i in range(FB)]
    bf1_sb = bias_chunks(fA, "bf1", FB)
    pfA = open_pool(name="pfA", bufs=3, space="PSUM", side="left")
    for fb in range(FB):
        ps = pfA.tile([P, 512], F32, name="ps_f1", tag="ps_f1")
        for kc in range(KC):
            nc.tensor.matmul(ps[:], lhsT=wf1_all[:, kc, fb * P:(fb + 1) * P],
                             rhs=a2T[kc][:, :],
                             start=(kc == 0), stop=(kc == KC - 1))
        nc.vector.tensor_scalar(out=f1T[fb][:], in0=ps[:],
                                scalar1=bf1_sb[:, fb:fb + 1], scalar2=0.0,
                                op0=ALU.add, op1=ALU.max)
    close_pool(pfA)
    close_pool(fw)
    close_pool(a2p)

    pfB = open_pool(name="pfB", bufs=1, space="PSUM", side="left")
    fB = open_pool(name="fB", bufs=1, side="right")
    ps_rb = [pfB.tile([P, D], F32, name=f"ps_rb{i}", tag=f"ps_rb{i}")
             for i in range(RB)]
    for fb in range(FB):
        wf2_fb = fB.tile([P, D], BF16, name="wf2s", tag="wf2s", bufs=3)
        nc.sync.dma_start(out=wf2_fb[:], in_=d["Wf2"][fb * P:(fb + 1) * P, :])
        for rb in range(RB):
            for nt in range(2):
                nc.tensor.matmul(ps_rb[rb][:, nt * 512:(nt + 1) * 512],
                                 lhsT=f1T[fb][:, rb * P:(rb + 1) * P],
                                 rhs=wf2_fb[:, nt * 512:(nt + 1) * 512],
                                 start=(fb == 0), stop=(fb == FB - 1))
    y_sb6 = [fB.tile([P, D], F32, name=f"y6{i}", tag=f"y6{i}")
             for i in range(RB)]
    for rb in range(RB):
        nc.sync.dma_start(out=y_sb6[rb][:], in_=d["y_rows"][rb * P:(rb + 1) * P, :])
    bf2_b = bcast_row(fB, "bf2")
    g3_b = bcast_row(fB, "ln3_g")
    be3_b = bcast_row(fB, "ln3_b")
    for rb in range(RB):
        ff = fB.tile([P, D], F32, name="ff", tag="ff", bufs=2)
        nc.vector.tensor_add(out=ff[:], in0=ps_rb[rb][:], in1=y_sb6[rb][:])
        nc.vector.tensor_add(out=ff[:], in0=ff[:], in1=bf2_b[:])
        _ln_rows(nc, fB, ff[:], eps_sb, g3_b[:], be3_b[:])
        nc.sync.dma_start(out=out_d[rb * P:(rb + 1) * P, :], in_=ff[:])
    close_pool(fB)
    close_pool(pfB)
    close_pool(fA)
    close_pool(dramp)
    close_pool(const)


def _row_idx(j):
    return np.concatenate(
        [np.arange(512 * rc + 128 * j, 512 * rc + 128 * j + 128)
         for rc in range(4)])


def _prep_host(inputs):
    f32 = lambda a: np.ascontiguousarray(np.asarray(a, np.float32))
    bf = lambda a: np.ascontiguousarray(
        np.asarray(a, np.float32).astype(ml_dtypes.bfloat16))
    x = f32(inputs["x"])
    y = f32(inputs["y"])
    mask = np.asarray(inputs["y_mask"]).astype(np.float32)
    # diagonal-block masks: mask4[ky, i, r] = mask[r, 128*i + ky]
    m4 = mask[0:512, 0:512].reshape(512, 4, 128).transpose(2, 1, 0)
    Wq = f32(inputs["Wq"])   # [H, D, DK]
    Wk = f32(inputs["Wk"])
    Wv = f32(inputs["Wv"])
    def chunkP(a):
        """[C*P, F] -> [P, C*F] so each partition's data is contiguous."""
        cp, f = a.shape
        return np.ascontiguousarray(
            a.reshape(cp // P, P, f).transpose(1, 0, 2).reshape(P, -1))

    shared = {
        "mask4": bf(m4),
        "b1": f32(inputs["b1"]),
        "ln1_g": f32(inputs["ln1_g"]), "ln1_b": f32(inputs["ln1_b"]),
        "W2sum": bf(f32(inputs["W2"]).reshape(H, D, D).sum(0)),
        "b2": f32(inputs["b2"]),
        "ln2_g": f32(inputs["ln2_g"]), "ln2_b": f32(inputs["ln2_b"]),
        "Wf1": chunkP(bf(inputs["Wf1"])),
        "bf1": f32(inputs["bf1"]),
        "Wf2": bf(inputs["Wf2"]),
        "bf2": f32(inputs["bf2"]),
        "ln3_g": f32(inputs["ln3_g"]), "ln3_b": f32(inputs["ln3_b"]),
    }
    in_maps = []
    for c in range(NCORES):
        b, j = c // 4, c % 4
        hh = slice(4 * j, 4 * j + 4)
        ridx = _row_idx(j)
        in_maps.append({
            "yT": bf(y[b].T),
            "wq": chunkP(bf(Wq[hh].transpose(1, 0, 2).reshape(D, 256) * SCALE)),
            "wk": chunkP(bf(Wk[hh].transpose(1, 0, 2).reshape(D, 256))),
            "wv": chunkP(bf(Wv[hh].transpose(1, 0, 2).reshape(D, 256))),
            "bq_s": f32(inputs["bq"])[hh].reshape(256) * np.float32(SCALE),
            "bk_f": f32(inputs["bk"])[hh].reshape(256),
            "bv_f": f32(inputs["bv"])[hh].reshape(256),
            "w1loc": chunkP(bf(f32(inputs["W1"])[256 * j:256 * (j + 1), :])),
            "y_rows": np.ascontiguousarray(y[b][ridx]),
            "xT": bf(x[b].T),
            "x_tm": bf(x[b]),
            **shared,
        })
    return in_maps


def kernel(**inputs):
    if "nc" not in _cached:
        _cached["nc"] = build_nc()
    nc = _cached["nc"]
    in_maps = _prep_host(inputs)
    res = run_bass_kernel_spmd(nc, in_maps, core_ids=list(range(NCORES)))
    out = np.zeros((B, S, D), np.float32)
    for c in range(NCORES):
        b, j = c // 4, c % 4
        out[b, _row_idx(j)] = res.results[c]["out"]
    return out

